# revision 7
# baseline (speedup 1.0000x reference)
"""Self-contained Trainium2 Bass kernel for nn_MultiHeadAttention.

Strategy (8 cores = 4 batches x 2 head-halves, 8 heads/core):
 - Everything computed in transposed "S^T" layout [k, q] so no on-chip
   transposes are needed anywhere:
     Q^T,K^T [dk, L] projections from host-transposed q^T,k^T inputs
     V [L, dv] projection from host-transposed v^T (lhsT = v^T blocks)
     S^T tile = (K^T block)^T @ Q^T chunk        (C=64, N=512, float32r)
     mask:  S^T += (-1e30*I) @ cmT  via PSUM accumulation (cm = 1-mask^T, bf16)
     Em^T  = exp(S^T * 1/8)  on ScalarE (masked entries -> exact 0)
     O^T  += [V_h | 1]^T @ Em^T  -> row 64 = softmax denominators (free)
     r = 1/sums;  R = ones^T @ r broadcast;  P^T = Em^T * R (DVE, in place)
     out_p = (O^T_norm blocks)^T @ Wo_half     (partial, host sums halves)
 - attn is written transposed [h, k, q] per core; host transposes back.
"""

import os
import sys

sys.path.insert(0, "/opt/trn_rl_repo")

import numpy as np
import ml_dtypes

B, L, D = 4, 2048, 1024
H, DK, DV = 16, 64, 64
HPC = 8  # heads per core
QC = 512  # q-chunk size
NQC = L // QC  # 4
NKT = L // 128  # 16 k-tiles

_NC = None
LAST_RESULT = None


def _build():
    import concourse.tile as tile
    from concourse import bacc, mybir

    F32 = mybir.dt.float32
    F32R = mybir.dt.float32r
    BF16 = mybir.dt.bfloat16
    Act = mybir.ActivationFunctionType

    nc = bacc.Bacc(None, target_bir_lowering=False)

    qt = nc.declare_dram_parameter("qt", [D, L], F32R, isOutput=False)
    kt = nc.declare_dram_parameter("kt", [D, L], F32R, isOutput=False)
    vt = nc.declare_dram_parameter("vt", [D, L], F32R, isOutput=False)
    cmt = nc.declare_dram_parameter("cmt", [L, L], BF16, isOutput=False)
    wq = nc.declare_dram_parameter("wq", [D, 512], F32R, isOutput=False)
    wk = nc.declare_dram_parameter("wk", [D, 512], F32R, isOutput=False)
    wv = nc.declare_dram_parameter("wv", [D, 512], F32R, isOutput=False)
    wo = nc.declare_dram_parameter("wo", [512, D], F32R, isOutput=False)
    bq = nc.declare_dram_parameter("bq", [128, 4], F32, isOutput=False)
    bk = nc.declare_dram_parameter("bk", [128, 4], F32, isOutput=False)
    bv = nc.declare_dram_parameter("bv", [1, 512], F32, isOutput=False)
    negi = nc.declare_dram_parameter("negi", [128, 128], BF16, isOutput=False)
    onescol = nc.declare_dram_parameter("onescol", [128, 8, 1], F32R, isOutput=False)

    attnT = nc.declare_dram_parameter("attnT", [HPC, L, L], F32, isOutput=True)
    outp = nc.declare_dram_parameter("outp", [L, D], F32, isOutput=True)

    with tile.TileContext(nc) as tc:
        with (
            tc.tile_pool(name="persist", bufs=1) as pp,
            tc.tile_pool(name="const", bufs=1) as cp,
        ):
            qt_sb = [pp.tile([128, L], F32R, name=f"qt_sb{m}") for m in range(4)]
            kt_sb = [pp.tile([128, L], F32R, name=f"kt_sb{m}") for m in range(4)]
            vext = [pp.tile([128, HPC, 65], F32R, name=f"vext{t}") for t in range(16)]
            ones_sb = cp.tile([1, 128], F32)
            nc.vector.memset(ones_sb[:], 1.0)
            negi_sb = cp.tile([128, 128], BF16)
            nc.sync.dma_start(out=negi_sb[:], in_=negi[:])

            # ---------------- Phase A: projections ----------------
            with (
                tc.tile_pool(name="wpool", bufs=1) as wp,
                tc.tile_pool(name="xstream", bufs=2) as xp,
                tc.tile_pool(name="biasp", bufs=1) as bp,
                tc.tile_pool(name="psA", bufs=4, space="PSUM") as psA,
            ):
                wq_sb = wp.tile([128, 8, 512], F32R)
                nc.sync.dma_start(
                    out=wq_sb[:], in_=wq[:].rearrange("(t p) j -> p t j", p=128)
                )
                wk_sb = wp.tile([128, 8, 512], F32R)
                nc.sync.dma_start(
                    out=wk_sb[:], in_=wk[:].rearrange("(t p) j -> p t j", p=128)
                )
                wv_sb = wp.tile([128, 8, 512], F32R)
                nc.sync.dma_start(
                    out=wv_sb[:], in_=wv[:].rearrange("(t p) j -> p t j", p=128)
                )
                bq_sb = bp.tile([128, 4], F32)
                nc.sync.dma_start(out=bq_sb[:], in_=bq[:])
                bk_sb = bp.tile([128, 4], F32)
                nc.sync.dma_start(out=bk_sb[:], in_=bk[:])
                bv_sb = bp.tile([1, 512], F32)
                nc.sync.dma_start(out=bv_sb[:], in_=bv[:])

                for src, wsb, bsb, dst in (
                    (qt, wq_sb, bq_sb, qt_sb),
                    (kt, wk_sb, bk_sb, kt_sb),
                ):
                    for l in range(4):
                        xs = xp.tile([128, 8, 512], F32R, tag="xs", name=f"xs_{l}")
                        nc.sync.dma_start(
                            out=xs[:],
                            in_=src[:].rearrange("(t p) j -> p t j", p=128)[
                                :, :, l * 512 : (l + 1) * 512
                            ],
                        )
                        for m in range(4):
                            ps = psA.tile([128, 512], F32, tag="pa", name=f"pa{l}{m}")
                            for t in range(8):
                                nc.tensor.matmul(
                                    ps[:],
                                    wsb[:, t, m * 128 : (m + 1) * 128],
                                    xs[:, t, :],
                                    start=(t == 0),
                                    stop=(t == 7),
                                )
                            nc.scalar.activation(
                                dst[m][:, l * 512 : (l + 1) * 512],
                                ps[:],
                                Act.Identity,
                                bias=bsb[:, m : m + 1],
                            )
                # V projection: natural [L, dv] with ones column per head
                for l in range(4):
                    xs = xp.tile([128, 8, 512], F32R, tag="xs", name=f"xsv_{l}")
                    nc.sync.dma_start(
                        out=xs[:],
                        in_=vt[:].rearrange("(t p) j -> p t j", p=128)[
                            :, :, l * 512 : (l + 1) * 512
                        ],
                    )
                    for ls in range(4):
                        lt = l * 4 + ls
                        ps = psA.tile([128, 512], F32, tag="pa", name=f"pv{lt}")
                        for t in range(8):
                            nc.tensor.matmul(
                                ps[:],
                                xs[:, t, ls * 128 : (ls + 1) * 128],
                                wv_sb[:, t, :],
                                start=(t == 0),
                                stop=False,
                            )
                        nc.tensor.matmul(
                            ps[:], ones_sb[:], bv_sb[:], start=False, stop=True
                        )
                        nc.sync.dma_start(
                            out=vext[lt][:, :, 64:65],
                            in_=onescol[:],
                        )
                        for h in range(HPC):
                            nc.scalar.copy(
                                vext[lt][:, h, 0:64], ps[:, h * 64 : (h + 1) * 64]
                            )

            # ---------------- Phase B: attention ----------------
            with (
                tc.tile_pool(name="empool", bufs=5) as ep,
                tc.tile_pool(name="cmpool", bufs=4) as cmp_,
                tc.tile_pool(name="otpool", bufs=1) as op_,
                tc.tile_pool(name="wopool", bufs=1) as wop,
                tc.tile_pool(name="smallp", bufs=2) as sp,
                tc.tile_pool(name="psS", bufs=2, space="PSUM") as psS,
                tc.tile_pool(name="psO", bufs=2, space="PSUM") as psO,
                tc.tile_pool(name="psR", bufs=2, space="PSUM") as psR,
            ):
                wo_sb = wop.tile([128, 4, D], F32R)
                nc.sync.dma_start(
                    out=wo_sb[:], in_=wo[:].rearrange("(t p) j -> p t j", p=128)
                )
                for qc in range(NQC):
                    q0 = qc * QC
                    cm_t = []
                    for g in range(4):
                        t_ = cmp_.tile(
                            [128, 4, QC], BF16, tag="cm", name=f"cm_{qc}_{g}"
                        )
                        nc.sync.dma_start(
                            out=t_[:],
                            in_=cmt[:].rearrange("(kt p) j -> p kt j", p=128)[
                                :, 4 * g : 4 * g + 4, q0 : q0 + QC
                            ],
                        )
                        cm_t.append(t_)
                    ot_sb = [
                        op_.tile([128, QC], F32R, tag=f"otsb{t}", name=f"ot_{qc}_{t}")
                        for t in range(4)
                    ]
                    for h in range(HPC):
                        m, r0 = h // 2, (h % 2) * 64
                        ot_ps = psO.tile([65, QC], F32, tag="ot", name=f"otp_{qc}_{h}")
                        em_t = [
                            ep.tile([128, 2048], F32R, tag="em", name=f"em{qc}_{h}_{g}")
                            for g in range(4)
                        ]
                        for jj in range(8):  # kt pairs
                            s_ps = psS.tile(
                                [128, 1024], F32, tag="s", name=f"s{qc}_{h}_{jj}"
                            )
                            for i in range(2):
                                ikt = jj * 2 + i
                                sl = s_ps[:, i * 512 : (i + 1) * 512]
                                nc.tensor.matmul(
                                    sl,
                                    negi_sb[:],
                                    cm_t[ikt // 4][:, ikt % 4, :],
                                    start=True,
                                    stop=False,
                                )
                                nc.tensor.matmul(
                                    sl,
                                    kt_sb[m][r0 : r0 + 64, ikt * 128 : (ikt + 1) * 128],
                                    qt_sb[m][r0 : r0 + 64, q0 : q0 + QC],
                                    start=False,
                                    stop=True,
                                )
                            g, hf = jj // 2, jj % 2
                            emsl = em_t[g][:, hf * 1024 : (hf + 1) * 1024]
                            nc.scalar.activation(emsl, s_ps[:], Act.Exp, scale=0.125)
                            for i in range(2):
                                ikt = jj * 2 + i
                                nc.tensor.matmul(
                                    ot_ps[:],
                                    vext[ikt][:, h, :],
                                    em_t[g][:, (2 * hf + i) * 512 : (2 * hf + i + 1) * 512],
                                    start=(ikt == 0),
                                    stop=(ikt == NKT - 1),
                                )
                        r_sb = sp.tile([1, QC], F32, tag="r", name=f"r{qc}_{h}")
                        nc.vector.reciprocal(r_sb[:], ot_ps[64:65, :])
                        rbc = psR.tile([128, QC], F32, tag="rbc", name=f"rb{qc}_{h}")
                        nc.tensor.matmul(rbc[:], ones_sb[:], r_sb[:], start=True, stop=True)
                        rbc_sb = sp.tile([128, QC], F32, tag="rsb", name=f"rs{qc}_{h}")
                        nc.scalar.copy(rbc_sb[:], rbc[:])
                        # normalize O^T into ot_sb (f32r out)
                        nc.vector.tensor_mul(
                            ot_sb[m][r0 : r0 + 64, :], ot_ps[0:64, :], rbc_sb[0:64, :]
                        )
                        # normalize P^T in place, then DMA to attnT
                        for g in range(4):
                            for i in range(4):
                                nc.vector.tensor_mul(
                                    em_t[g][:, i * 512 : (i + 1) * 512],
                                    em_t[g][:, i * 512 : (i + 1) * 512].bitcast(F32),
                                    rbc_sb[:],
                                )
                            nc.sync.dma_start(
                                out=attnT[h].rearrange("(kt p) j -> p kt j", p=128)[
                                    :, 4 * g : 4 * g + 4, q0 : q0 + QC
                                ],
                                in_=em_t[g][:]
                                .bitcast(F32)
                                .rearrange("p (kt j) -> p kt j", j=512),
                            )
                    # out projection for this q-chunk (partial out, bo on host)
                    for qs in range(4):
                        ps = psS.tile([128, 1024], F32, tag="s", name=f"op{qc}_{qs}")
                        for dch in range(2):
                            for t in range(4):
                                nc.tensor.matmul(
                                    ps[:, dch * 512 : (dch + 1) * 512],
                                    ot_sb[t][:, qs * 128 : (qs + 1) * 128],
                                    wo_sb[:, t, dch * 512 : (dch + 1) * 512],
                                    start=(t == 0),
                                    stop=(t == 3),
                                )
                        ost = sp.tile([128, 1024], F32, tag="ost", name=f"os{qc}_{qs}")
                        nc.scalar.copy(ost[:], ps[:])
                        nc.sync.dma_start(
                            out=outp[q0 + qs * 128 : q0 + (qs + 1) * 128, :],
                            in_=ost[:],
                        )

    nc.compile()
    return nc


def _get_nc():
    global _NC
    if _NC is None:
        _NC = _build()
    return _NC


def kernel(q, k, v, mask, Wq, bq, Wk, bk, Wv, bv, Wo, bo):
    global LAST_RESULT
    from concourse.bass_utils import run_bass_kernel_spmd

    nc = _get_nc()
    q = np.asarray(q, np.float32)
    k = np.asarray(k, np.float32)
    v = np.asarray(v, np.float32)
    mask = np.asarray(mask)
    Wq = np.asarray(Wq, np.float32)
    Wk = np.asarray(Wk, np.float32)
    Wv = np.asarray(Wv, np.float32)
    Wo = np.asarray(Wo, np.float32)

    negi = np.zeros((128, 128), np.float32)
    np.fill_diagonal(negi, -1e30)
    negi = negi.astype(ml_dtypes.bfloat16)

    qT = [np.ascontiguousarray(q[b].T) for b in range(B)]
    kT = [np.ascontiguousarray(k[b].T) for b in range(B)]
    vT = [np.ascontiguousarray(v[b].T) for b in range(B)]
    cmT = [(mask[b] == 0).T.astype(ml_dtypes.bfloat16) for b in range(B)]

    in_maps = []
    for c in range(8):
        b, half = divmod(c, 2)
        hs = slice(half * 512, (half + 1) * 512)
        in_maps.append(
            dict(
                qt=qT[b],
                kt=kT[b],
                vt=vT[b],
                cmt=cmT[b],
                wq=np.ascontiguousarray(Wq[:, hs]),
                wk=np.ascontiguousarray(Wk[:, hs]),
                wv=np.ascontiguousarray(Wv[:, hs]),
                wo=np.ascontiguousarray(Wo[hs, :]),
                bq=np.ascontiguousarray(
                    np.asarray(bq, np.float32)[hs].reshape(4, 128).T
                ),
                bk=np.ascontiguousarray(
                    np.asarray(bk, np.float32)[hs].reshape(4, 128).T
                ),
                bv=np.asarray(bv, np.float32)[hs].reshape(1, 512),
                negi=negi,
                onescol=np.ones((128, 8, 1), np.float32),
            )
        )

    trace = bool(int(os.environ.get("BASS_KERNEL_TRACE", "0")))
    if trace:
        try:
            try:
                from antenv.axon_hooks import get_axon_ntff_profile_hook
            except ImportError:
                import types

                import antenv

                mod = types.ModuleType("antenv.axon_hooks")
                mod._HOOK = None

                def _set(hook, _m=mod):
                    _m._HOOK = hook

                def _get(_m=mod):
                    return _m._HOOK

                mod.set_axon_ntff_profile_hook = _set
                mod.get_axon_ntff_profile_hook = _get
                sys.modules["antenv.axon_hooks"] = mod
                antenv.axon_hooks = mod
                from antenv.axon_hooks import get_axon_ntff_profile_hook

            if get_axon_ntff_profile_hook() is None:
                if "/root/.axon_site" not in sys.path:
                    sys.path.insert(0, "/root/.axon_site")
                from trn_agent_boot.trn_boot import _ntff_profile_via_ctypes

                sys.modules["antenv.axon_hooks"].set_axon_ntff_profile_hook(
                    _ntff_profile_via_ctypes("/opt/axon/libaxon_pjrt.so")
                )
        except Exception as e:  # tracing is best-effort
            print(f"trace hook setup failed: {e}", file=sys.stderr)
    res = run_bass_kernel_spmd(
        nc, in_maps, core_ids=list(range(8)), trace=trace
    )
    LAST_RESULT = res

    out = np.empty((B, L, D), np.float32)
    attn = np.empty((B, H, L, L), np.float32)
    for c in range(8):
        b, half = divmod(c, 2)
        r = res.results[c]
        attn[b, half * HPC : (half + 1) * HPC] = r["attnT"].transpose(0, 2, 1)
        if half == 0:
            out[b] = r["outp"]
        else:
            out[b] += r["outp"]
    out += np.asarray(bo, np.float32)
    return out, attn


# revision 8
# speedup vs baseline: 1.0026x; 1.0026x over previous
"""Self-contained Trainium2 Bass kernel for nn_MultiHeadAttention.

Strategy (8 cores = 4 batches x 2 head-halves, 8 heads/core):
 - Everything computed in transposed "S^T" layout [k, q] so no on-chip
   transposes are needed anywhere:
     Q^T,K^T [dk, L] projections from host-transposed q^T,k^T inputs
     V [L, dv] projection from host-transposed v^T (lhsT = v^T blocks)
     S^T tile = (K^T block)^T @ Q^T chunk        (C=64, N=512, float32r)
     mask:  S^T += (-1e30*I) @ cmT  via PSUM accumulation (cm = 1-mask^T, bf16)
     Em^T  = exp(S^T * 1/8)  on ScalarE (masked entries -> exact 0)
     O^T  += [V_h | 1]^T @ Em^T  -> row 64 = softmax denominators (free)
     r = 1/sums;  R = ones^T @ r broadcast;  P^T = Em^T * R (DVE, in place)
     out_p = (O^T_norm blocks)^T @ Wo_half     (partial, host sums halves)
 - attn is written transposed [h, k, q] per core; host transposes back.
"""

import os
import sys

sys.path.insert(0, "/opt/trn_rl_repo")

import numpy as np
import ml_dtypes

B, L, D = 4, 2048, 1024
H, DK, DV = 16, 64, 64
HPC = 8  # heads per core
QC = 512  # q-chunk size
NQC = L // QC  # 4
NKT = L // 128  # 16 k-tiles

_NC = None
LAST_RESULT = None


def _build():
    import concourse.tile as tile
    from concourse import bacc, mybir

    F32 = mybir.dt.float32
    F32R = mybir.dt.float32r
    BF16 = mybir.dt.bfloat16
    Act = mybir.ActivationFunctionType

    nc = bacc.Bacc(None, target_bir_lowering=False)

    qt = nc.declare_dram_parameter("qt", [D, L], F32R, isOutput=False)
    kt = nc.declare_dram_parameter("kt", [D, L], F32R, isOutput=False)
    vt = nc.declare_dram_parameter("vt", [D, L], F32R, isOutput=False)
    cmt = nc.declare_dram_parameter("cmt", [L, L], BF16, isOutput=False)
    wq = nc.declare_dram_parameter("wq", [D, 512], F32R, isOutput=False)
    wk = nc.declare_dram_parameter("wk", [D, 512], F32R, isOutput=False)
    wv = nc.declare_dram_parameter("wv", [D, 512], F32R, isOutput=False)
    wo = nc.declare_dram_parameter("wo", [512, D], F32R, isOutput=False)
    bq = nc.declare_dram_parameter("bq", [128, 4], F32, isOutput=False)
    bk = nc.declare_dram_parameter("bk", [128, 4], F32, isOutput=False)
    bv = nc.declare_dram_parameter("bv", [1, 512], F32, isOutput=False)
    negi = nc.declare_dram_parameter("negi", [128, 128], BF16, isOutput=False)
    onescol = nc.declare_dram_parameter("onescol", [128, 8, 1], F32R, isOutput=False)

    attnT = nc.declare_dram_parameter("attnT", [HPC, L, L], F32, isOutput=True)
    outp = nc.declare_dram_parameter("outp", [L, D], F32, isOutput=True)

    with tile.TileContext(nc) as tc:
        with (
            tc.tile_pool(name="persist", bufs=1) as pp,
            tc.tile_pool(name="const", bufs=1) as cp,
        ):
            qt_sb = [pp.tile([128, L], F32R, name=f"qt_sb{m}") for m in range(4)]
            kt_sb = [pp.tile([128, L], F32R, name=f"kt_sb{m}") for m in range(4)]
            vext = [pp.tile([128, HPC, 65], F32R, name=f"vext{t}") for t in range(16)]
            ones_sb = cp.tile([1, 128], F32)
            nc.vector.memset(ones_sb[:], 1.0)
            negi_sb = cp.tile([128, 128], BF16)
            nc.sync.dma_start(out=negi_sb[:], in_=negi[:])

            # ---------------- Phase A: projections ----------------
            with (
                tc.tile_pool(name="wpool", bufs=1) as wp,
                tc.tile_pool(name="xstream", bufs=2) as xp,
                tc.tile_pool(name="biasp", bufs=1) as bp,
                tc.tile_pool(name="psA", bufs=4, space="PSUM") as psA,
            ):
                wq_sb = wp.tile([128, 8, 512], F32R)
                nc.sync.dma_start(
                    out=wq_sb[:], in_=wq[:].rearrange("(t p) j -> p t j", p=128)
                )
                wk_sb = wp.tile([128, 8, 512], F32R)
                nc.sync.dma_start(
                    out=wk_sb[:], in_=wk[:].rearrange("(t p) j -> p t j", p=128)
                )
                wv_sb = wp.tile([128, 8, 512], F32R)
                nc.sync.dma_start(
                    out=wv_sb[:], in_=wv[:].rearrange("(t p) j -> p t j", p=128)
                )
                bq_sb = bp.tile([128, 4], F32)
                nc.sync.dma_start(out=bq_sb[:], in_=bq[:])
                bk_sb = bp.tile([128, 4], F32)
                nc.sync.dma_start(out=bk_sb[:], in_=bk[:])
                bv_sb = bp.tile([1, 512], F32)
                nc.sync.dma_start(out=bv_sb[:], in_=bv[:])

                for src, wsb, bsb, dst in (
                    (qt, wq_sb, bq_sb, qt_sb),
                    (kt, wk_sb, bk_sb, kt_sb),
                ):
                    for l in range(4):
                        xs = xp.tile([128, 8, 512], F32R, tag="xs", name=f"xs_{l}")
                        nc.sync.dma_start(
                            out=xs[:],
                            in_=src[:].rearrange("(t p) j -> p t j", p=128)[
                                :, :, l * 512 : (l + 1) * 512
                            ],
                        )
                        for m in range(4):
                            ps = psA.tile([128, 512], F32, tag="pa", name=f"pa{l}{m}")
                            for t in range(8):
                                nc.tensor.matmul(
                                    ps[:],
                                    wsb[:, t, m * 128 : (m + 1) * 128],
                                    xs[:, t, :],
                                    start=(t == 0),
                                    stop=(t == 7),
                                )
                            nc.scalar.activation(
                                dst[m][:, l * 512 : (l + 1) * 512],
                                ps[:],
                                Act.Identity,
                                bias=bsb[:, m : m + 1],
                            )
                # V projection: natural [L, dv] with ones column per head
                for l in range(4):
                    xs = xp.tile([128, 8, 512], F32R, tag="xs", name=f"xsv_{l}")
                    nc.sync.dma_start(
                        out=xs[:],
                        in_=vt[:].rearrange("(t p) j -> p t j", p=128)[
                            :, :, l * 512 : (l + 1) * 512
                        ],
                    )
                    for ls in range(4):
                        lt = l * 4 + ls
                        ps = psA.tile([128, 512], F32, tag="pa", name=f"pv{lt}")
                        for t in range(8):
                            nc.tensor.matmul(
                                ps[:],
                                xs[:, t, ls * 128 : (ls + 1) * 128],
                                wv_sb[:, t, :],
                                start=(t == 0),
                                stop=False,
                            )
                        nc.tensor.matmul(
                            ps[:], ones_sb[:], bv_sb[:], start=False, stop=True
                        )
                        nc.sync.dma_start(
                            out=vext[lt][:, :, 64:65],
                            in_=onescol[:],
                        )
                        for h in range(HPC):
                            nc.scalar.copy(
                                vext[lt][:, h, 0:64], ps[:, h * 64 : (h + 1) * 64]
                            )

            # ---------------- Phase B: attention ----------------
            with (
                tc.tile_pool(name="empool", bufs=5) as ep,
                tc.tile_pool(name="cmpool", bufs=4) as cmp_,
                tc.tile_pool(name="otpool", bufs=1) as op_,
                tc.tile_pool(name="wopool", bufs=1) as wop,
                tc.tile_pool(name="smallp", bufs=2) as sp,
                tc.tile_pool(name="psS", bufs=2, space="PSUM") as psS,
                tc.tile_pool(name="psO", bufs=2, space="PSUM") as psO,
                tc.tile_pool(name="psR", bufs=2, space="PSUM") as psR,
            ):
                wo_sb = wop.tile([128, 4, D], F32R)
                nc.sync.dma_start(
                    out=wo_sb[:], in_=wo[:].rearrange("(t p) j -> p t j", p=128)
                )
                for qc in range(NQC):
                    q0 = qc * QC
                    cm_t = []
                    for g in range(4):
                        t_ = cmp_.tile(
                            [128, 4, QC], BF16, tag="cm", name=f"cm_{qc}_{g}"
                        )
                        nc.sync.dma_start(
                            out=t_[:],
                            in_=cmt[:].rearrange("(kt p) j -> p kt j", p=128)[
                                :, 4 * g : 4 * g + 4, q0 : q0 + QC
                            ],
                        )
                        cm_t.append(t_)
                    ot_sb = [
                        op_.tile([128, QC], F32R, tag=f"otsb{t}", name=f"ot_{qc}_{t}")
                        for t in range(4)
                    ]
                    for h in range(HPC):
                        m, r0 = h // 2, (h % 2) * 64
                        ot_ps = psO.tile([65, QC], F32, tag="ot", name=f"otp_{qc}_{h}")
                        em_t = [
                            ep.tile([128, 2048], F32R, tag="em", name=f"em{qc}_{h}_{g}")
                            for g in range(4)
                        ]
                        for jj in range(8):  # kt pairs
                            s_ps = psS.tile(
                                [128, 1024], F32, tag="s", name=f"s{qc}_{h}_{jj}"
                            )
                            for i in range(2):
                                ikt = jj * 2 + i
                                sl = s_ps[:, i * 512 : (i + 1) * 512]
                                nc.tensor.matmul(
                                    sl,
                                    negi_sb[:],
                                    cm_t[ikt // 4][:, ikt % 4, :],
                                    start=True,
                                    stop=False,
                                )
                                nc.tensor.matmul(
                                    sl,
                                    kt_sb[m][r0 : r0 + 64, ikt * 128 : (ikt + 1) * 128],
                                    qt_sb[m][r0 : r0 + 64, q0 : q0 + QC],
                                    start=False,
                                    stop=True,
                                )
                            g, hf = jj // 2, jj % 2
                            emsl = em_t[g][:, hf * 1024 : (hf + 1) * 1024]
                            nc.scalar.activation(emsl, s_ps[:], Act.Exp, scale=0.125)
                            for i in range(2):
                                ikt = jj * 2 + i
                                nc.tensor.matmul(
                                    ot_ps[:],
                                    vext[ikt][:, h, :],
                                    em_t[g][:, (2 * hf + i) * 512 : (2 * hf + i + 1) * 512],
                                    start=(ikt == 0),
                                    stop=(ikt == NKT - 1),
                                )
                        sums_sb = sp.tile([1, QC], F32, tag="r", name=f"r{qc}_{h}")
                        nc.scalar.copy(sums_sb[:], ot_ps[64:65, :])
                        rbc = psR.tile([128, QC], F32, tag="rbc", name=f"rb{qc}_{h}")
                        nc.tensor.matmul(
                            rbc[:], ones_sb[:], sums_sb[:], start=True, stop=True
                        )
                        rbc_sb = sp.tile([128, QC], F32, tag="rsb", name=f"rs{qc}_{h}")
                        nc.vector.reciprocal(rbc_sb[:], rbc[:])
                        # normalize O^T into ot_sb (f32r out)
                        nc.vector.tensor_mul(
                            ot_sb[m][r0 : r0 + 64, :], ot_ps[0:64, :], rbc_sb[0:64, :]
                        )
                        # normalize P^T in place, then DMA to attnT
                        for g in range(4):
                            for i in range(4):
                                nc.vector.tensor_mul(
                                    em_t[g][:, i * 512 : (i + 1) * 512],
                                    em_t[g][:, i * 512 : (i + 1) * 512].bitcast(F32),
                                    rbc_sb[:],
                                )
                            nc.sync.dma_start(
                                out=attnT[h].rearrange("(kt p) j -> p kt j", p=128)[
                                    :, 4 * g : 4 * g + 4, q0 : q0 + QC
                                ],
                                in_=em_t[g][:]
                                .bitcast(F32)
                                .rearrange("p (kt j) -> p kt j", j=512),
                            )
                    # out projection for this q-chunk (partial out, bo on host)
                    for qs in range(4):
                        ps = psS.tile([128, 1024], F32, tag="s", name=f"op{qc}_{qs}")
                        for dch in range(2):
                            for t in range(4):
                                nc.tensor.matmul(
                                    ps[:, dch * 512 : (dch + 1) * 512],
                                    ot_sb[t][:, qs * 128 : (qs + 1) * 128],
                                    wo_sb[:, t, dch * 512 : (dch + 1) * 512],
                                    start=(t == 0),
                                    stop=(t == 3),
                                )
                        ost = sp.tile([128, 1024], F32, tag="ost", name=f"os{qc}_{qs}")
                        nc.scalar.copy(ost[:], ps[:])
                        nc.sync.dma_start(
                            out=outp[q0 + qs * 128 : q0 + (qs + 1) * 128, :],
                            in_=ost[:],
                        )

    nc.compile()
    return nc


def _get_nc():
    global _NC
    if _NC is None:
        _NC = _build()
    return _NC


def kernel(q, k, v, mask, Wq, bq, Wk, bk, Wv, bv, Wo, bo):
    global LAST_RESULT
    from concourse.bass_utils import run_bass_kernel_spmd

    nc = _get_nc()
    q = np.asarray(q, np.float32)
    k = np.asarray(k, np.float32)
    v = np.asarray(v, np.float32)
    mask = np.asarray(mask)
    Wq = np.asarray(Wq, np.float32)
    Wk = np.asarray(Wk, np.float32)
    Wv = np.asarray(Wv, np.float32)
    Wo = np.asarray(Wo, np.float32)

    negi = np.zeros((128, 128), np.float32)
    np.fill_diagonal(negi, -1e30)
    negi = negi.astype(ml_dtypes.bfloat16)

    qT = [np.ascontiguousarray(q[b].T) for b in range(B)]
    kT = [np.ascontiguousarray(k[b].T) for b in range(B)]
    vT = [np.ascontiguousarray(v[b].T) for b in range(B)]
    cmT = [(mask[b] == 0).T.astype(ml_dtypes.bfloat16) for b in range(B)]

    in_maps = []
    for c in range(8):
        b, half = divmod(c, 2)
        hs = slice(half * 512, (half + 1) * 512)
        in_maps.append(
            dict(
                qt=qT[b],
                kt=kT[b],
                vt=vT[b],
                cmt=cmT[b],
                wq=np.ascontiguousarray(Wq[:, hs]),
                wk=np.ascontiguousarray(Wk[:, hs]),
                wv=np.ascontiguousarray(Wv[:, hs]),
                wo=np.ascontiguousarray(Wo[hs, :]),
                bq=np.ascontiguousarray(
                    np.asarray(bq, np.float32)[hs].reshape(4, 128).T
                ),
                bk=np.ascontiguousarray(
                    np.asarray(bk, np.float32)[hs].reshape(4, 128).T
                ),
                bv=np.asarray(bv, np.float32)[hs].reshape(1, 512),
                negi=negi,
                onescol=np.ones((128, 8, 1), np.float32),
            )
        )

    trace = bool(int(os.environ.get("BASS_KERNEL_TRACE", "0")))
    if trace:
        try:
            try:
                from antenv.axon_hooks import get_axon_ntff_profile_hook
            except ImportError:
                import types

                import antenv

                mod = types.ModuleType("antenv.axon_hooks")
                mod._HOOK = None

                def _set(hook, _m=mod):
                    _m._HOOK = hook

                def _get(_m=mod):
                    return _m._HOOK

                mod.set_axon_ntff_profile_hook = _set
                mod.get_axon_ntff_profile_hook = _get
                sys.modules["antenv.axon_hooks"] = mod
                antenv.axon_hooks = mod
                from antenv.axon_hooks import get_axon_ntff_profile_hook

            if get_axon_ntff_profile_hook() is None:
                if "/root/.axon_site" not in sys.path:
                    sys.path.insert(0, "/root/.axon_site")
                from trn_agent_boot.trn_boot import _ntff_profile_via_ctypes

                sys.modules["antenv.axon_hooks"].set_axon_ntff_profile_hook(
                    _ntff_profile_via_ctypes("/opt/axon/libaxon_pjrt.so")
                )
        except Exception as e:  # tracing is best-effort
            print(f"trace hook setup failed: {e}", file=sys.stderr)
    res = run_bass_kernel_spmd(
        nc, in_maps, core_ids=list(range(8)), trace=trace
    )
    LAST_RESULT = res

    out = np.empty((B, L, D), np.float32)
    attn = np.empty((B, H, L, L), np.float32)
    for c in range(8):
        b, half = divmod(c, 2)
        r = res.results[c]
        attn[b, half * HPC : (half + 1) * HPC] = r["attnT"].transpose(0, 2, 1)
        if half == 0:
            out[b] = r["outp"]
        else:
            out[b] += r["outp"]
    out += np.asarray(bo, np.float32)
    return out, attn


# revision 9
# speedup vs baseline: 1.0630x; 1.0602x over previous
"""Self-contained Trainium2 Bass kernel for nn_MultiHeadAttention.

Strategy (8 cores = 4 batches x 2 head-halves, 8 heads/core):
 - Everything computed in transposed "S^T" layout [k, q] so no on-chip
   transposes are needed anywhere:
     Q^T,K^T [dk, L] projections from host-transposed q^T,k^T inputs
     V [L, dv] projection from host-transposed v^T (lhsT = v^T blocks)
     S^T tile = (K^T block)^T @ Q^T chunk        (C=64, N=512, float32r)
     mask:  S^T += (-1e30*I) @ cmT  via PSUM accumulation (cm = 1-mask^T, bf16)
     Em^T  = exp(S^T * 1/8)  on ScalarE (masked entries -> exact 0)
     O^T  += [V_h | 1]^T @ Em^T  -> row 64 = softmax denominators (free)
     r = 1/sums;  R = ones^T @ r broadcast;  P^T = Em^T * R (DVE, in place)
     out_p = (O^T_norm blocks)^T @ Wo_half     (partial, host sums halves)
 - attn is written transposed [h, k, q] per core; host transposes back.
"""

import os
import sys

sys.path.insert(0, "/opt/trn_rl_repo")

import numpy as np
import ml_dtypes

B, L, D = 4, 2048, 1024
H, DK, DV = 16, 64, 64
HPC = 8  # heads per core
QC = 512  # q-chunk size
NQC = L // QC  # 4
NKT = L // 128  # 16 k-tiles

_NC = None
LAST_RESULT = None


def _build():
    import concourse.tile as tile
    from concourse import bacc, mybir

    F32 = mybir.dt.float32
    F32R = mybir.dt.float32r
    BF16 = mybir.dt.bfloat16
    Act = mybir.ActivationFunctionType

    nc = bacc.Bacc(None, target_bir_lowering=False)

    qt = nc.declare_dram_parameter("qt", [D, L], F32R, isOutput=False)
    kt = nc.declare_dram_parameter("kt", [D, L], F32R, isOutput=False)
    vt = nc.declare_dram_parameter("vt", [D, L], F32R, isOutput=False)
    cmt = nc.declare_dram_parameter("cmt", [L, L], BF16, isOutput=False)
    wq = nc.declare_dram_parameter("wq", [D, 512], F32R, isOutput=False)
    wk = nc.declare_dram_parameter("wk", [D, 512], F32R, isOutput=False)
    wv = nc.declare_dram_parameter("wv", [D, 512], F32R, isOutput=False)
    wo = nc.declare_dram_parameter("wo", [512, D], F32R, isOutput=False)
    bq = nc.declare_dram_parameter("bq", [128, 4], F32, isOutput=False)
    bk = nc.declare_dram_parameter("bk", [128, 4], F32, isOutput=False)
    bv = nc.declare_dram_parameter("bv", [1, 512], F32, isOutput=False)
    negi = nc.declare_dram_parameter("negi", [128, 128], BF16, isOutput=False)
    onescol = nc.declare_dram_parameter("onescol", [128, 8, 1], F32R, isOutput=False)

    attnT = nc.declare_dram_parameter("attnT", [HPC, L, L], F32, isOutput=True)
    outp = nc.declare_dram_parameter("outp", [L, D], F32, isOutput=True)

    with tile.TileContext(nc) as tc:
        with (
            tc.tile_pool(name="persist", bufs=1) as pp,
            tc.tile_pool(name="const", bufs=1) as cp,
        ):
            qt_sb = [pp.tile([128, L], F32R, name=f"qt_sb{m}") for m in range(4)]
            kt_sb = [pp.tile([128, L], F32R, name=f"kt_sb{m}") for m in range(4)]
            vext = [pp.tile([128, HPC, 65], F32R, name=f"vext{t}") for t in range(16)]
            ones_sb = cp.tile([1, 128], F32)
            nc.vector.memset(ones_sb[:], 1.0)
            negi_sb = cp.tile([128, 128], BF16)
            nc.sync.dma_start(out=negi_sb[:], in_=negi[:])

            # ---------------- Phase A: projections ----------------
            with (
                tc.tile_pool(name="wpool", bufs=1) as wp,
                tc.tile_pool(name="xstream", bufs=2) as xp,
                tc.tile_pool(name="biasp", bufs=1) as bp,
                tc.tile_pool(name="psA", bufs=4, space="PSUM") as psA,
            ):
                wq_sb = wp.tile([128, 8, 512], F32R)
                nc.sync.dma_start(
                    out=wq_sb[:], in_=wq[:].rearrange("(t p) j -> p t j", p=128)
                )
                wk_sb = wp.tile([128, 8, 512], F32R)
                nc.sync.dma_start(
                    out=wk_sb[:], in_=wk[:].rearrange("(t p) j -> p t j", p=128)
                )
                wv_sb = wp.tile([128, 8, 512], F32R)
                nc.sync.dma_start(
                    out=wv_sb[:], in_=wv[:].rearrange("(t p) j -> p t j", p=128)
                )
                bq_sb = bp.tile([128, 4], F32)
                nc.sync.dma_start(out=bq_sb[:], in_=bq[:])
                bk_sb = bp.tile([128, 4], F32)
                nc.sync.dma_start(out=bk_sb[:], in_=bk[:])
                bv_sb = bp.tile([1, 512], F32)
                nc.sync.dma_start(out=bv_sb[:], in_=bv[:])

                for src, wsb, bsb, dst in (
                    (qt, wq_sb, bq_sb, qt_sb),
                    (kt, wk_sb, bk_sb, kt_sb),
                ):
                    for l in range(4):
                        xs = xp.tile([128, 8, 512], F32R, tag="xs", name=f"xs_{l}")
                        nc.sync.dma_start(
                            out=xs[:],
                            in_=src[:].rearrange("(t p) j -> p t j", p=128)[
                                :, :, l * 512 : (l + 1) * 512
                            ],
                        )
                        for m in range(4):
                            ps = psA.tile([128, 512], F32, tag="pa", name=f"pa{l}{m}")
                            for t in range(8):
                                nc.tensor.matmul(
                                    ps[:],
                                    wsb[:, t, m * 128 : (m + 1) * 128],
                                    xs[:, t, :],
                                    start=(t == 0),
                                    stop=(t == 7),
                                )
                            nc.scalar.activation(
                                dst[m][:, l * 512 : (l + 1) * 512],
                                ps[:],
                                Act.Identity,
                                bias=bsb[:, m : m + 1],
                            )
                # V projection: natural [L, dv] with ones column per head
                for l in range(4):
                    xs = xp.tile([128, 8, 512], F32R, tag="xs", name=f"xsv_{l}")
                    nc.sync.dma_start(
                        out=xs[:],
                        in_=vt[:].rearrange("(t p) j -> p t j", p=128)[
                            :, :, l * 512 : (l + 1) * 512
                        ],
                    )
                    for ls in range(4):
                        lt = l * 4 + ls
                        ps = psA.tile([128, 512], F32, tag="pa", name=f"pv{lt}")
                        for t in range(8):
                            nc.tensor.matmul(
                                ps[:],
                                xs[:, t, ls * 128 : (ls + 1) * 128],
                                wv_sb[:, t, :],
                                start=(t == 0),
                                stop=False,
                            )
                        nc.tensor.matmul(
                            ps[:], ones_sb[:], bv_sb[:], start=False, stop=True
                        )
                        nc.sync.dma_start(
                            out=vext[lt][:, :, 64:65],
                            in_=onescol[:],
                        )
                        for h in range(HPC):
                            nc.scalar.copy(
                                vext[lt][:, h, 0:64], ps[:, h * 64 : (h + 1) * 64]
                            )

            # ---------------- Phase B: attention ----------------
            with (
                tc.tile_pool(name="empool", bufs=5) as ep,
                tc.tile_pool(name="cmpool", bufs=4) as cmp_,
                tc.tile_pool(name="otpool", bufs=2) as op_,
                tc.tile_pool(name="wopool", bufs=1) as wop,
                tc.tile_pool(name="smallp", bufs=2) as sp,
                tc.tile_pool(name="psS", bufs=2, space="PSUM") as psS,
                tc.tile_pool(name="psO", bufs=2, space="PSUM") as psO,
                tc.tile_pool(name="psR", bufs=1, space="PSUM") as psR,
                tc.tile_pool(name="psOP", bufs=1, space="PSUM") as psOP,
            ):
                wo_sb = wop.tile([128, 4, D], F32R)
                nc.sync.dma_start(
                    out=wo_sb[:], in_=wo[:].rearrange("(t p) j -> p t j", p=128)
                )
                for qc in range(NQC):
                    q0 = qc * QC
                    cm_t = []
                    for g in range(4):
                        t_ = cmp_.tile(
                            [128, 4, QC], BF16, tag="cm", name=f"cm_{qc}_{g}"
                        )
                        nc.sync.dma_start(
                            out=t_[:],
                            in_=cmt[:].rearrange("(kt p) j -> p kt j", p=128)[
                                :, 4 * g : 4 * g + 4, q0 : q0 + QC
                            ],
                        )
                        cm_t.append(t_)
                    ot_sb = [
                        op_.tile([128, QC], F32R, tag=f"otsb{t}", name=f"ot_{qc}_{t}")
                        for t in range(4)
                    ]
                    for h in range(HPC):
                        m, r0 = h // 2, (h % 2) * 64
                        ot_ps = psO.tile([65, QC], F32, tag="ot", name=f"otp_{qc}_{h}")
                        em_t = [
                            ep.tile([128, 2048], F32R, tag="em", name=f"em{qc}_{h}_{g}")
                            for g in range(4)
                        ]
                        for jj in range(8):  # kt pairs
                            s_ps = psS.tile(
                                [128, 1024], F32, tag="s", name=f"s{qc}_{h}_{jj}"
                            )
                            for i in range(2):
                                ikt = jj * 2 + i
                                sl = s_ps[:, i * 512 : (i + 1) * 512]
                                nc.tensor.matmul(
                                    sl,
                                    negi_sb[:],
                                    cm_t[ikt // 4][:, ikt % 4, :],
                                    start=True,
                                    stop=False,
                                )
                                nc.tensor.matmul(
                                    sl,
                                    kt_sb[m][r0 : r0 + 64, ikt * 128 : (ikt + 1) * 128],
                                    qt_sb[m][r0 : r0 + 64, q0 : q0 + QC],
                                    start=False,
                                    stop=True,
                                )
                            g, hf = jj // 2, jj % 2
                            emsl = em_t[g][:, hf * 1024 : (hf + 1) * 1024]
                            nc.scalar.activation(emsl, s_ps[:], Act.Exp, scale=0.125)
                            for i in range(2):
                                ikt = jj * 2 + i
                                nc.tensor.matmul(
                                    ot_ps[:],
                                    vext[ikt][:, h, :],
                                    em_t[g][:, (2 * hf + i) * 512 : (2 * hf + i + 1) * 512],
                                    start=(ikt == 0),
                                    stop=(ikt == NKT - 1),
                                )
                        sums_sb = sp.tile([1, QC], F32, tag="r", name=f"r{qc}_{h}")
                        nc.scalar.copy(sums_sb[:], ot_ps[64:65, :])
                        rbc = psR.tile([128, QC], F32, tag="rbc", name=f"rb{qc}_{h}")
                        nc.tensor.matmul(
                            rbc[:], ones_sb[:], sums_sb[:], start=True, stop=True
                        )
                        rbc_sb = sp.tile([128, QC], F32, tag="rsb", name=f"rs{qc}_{h}")
                        nc.vector.reciprocal(rbc_sb[:], rbc[:])
                        # normalize O^T into ot_sb (f32r out)
                        nc.vector.tensor_mul(
                            ot_sb[m][r0 : r0 + 64, :], ot_ps[0:64, :], rbc_sb[0:64, :]
                        )
                        # normalize P^T in place, then DMA to attnT
                        for g in range(4):
                            for i in range(4):
                                nc.vector.tensor_mul(
                                    em_t[g][:, i * 512 : (i + 1) * 512],
                                    em_t[g][:, i * 512 : (i + 1) * 512].bitcast(F32),
                                    rbc_sb[:],
                                )
                            nc.sync.dma_start(
                                out=attnT[h].rearrange("(kt p) j -> p kt j", p=128)[
                                    :, 4 * g : 4 * g + 4, q0 : q0 + QC
                                ],
                                in_=em_t[g][:]
                                .bitcast(F32)
                                .rearrange("p (kt j) -> p kt j", j=512),
                            )
                    # out projection for this q-chunk (partial out, bo on host)
                    for qs in range(4):
                        for dch in range(2):
                            ps = psOP.tile(
                                [128, 512], F32, tag="sop", name=f"op{qc}_{qs}_{dch}"
                            )
                            for t in range(4):
                                nc.tensor.matmul(
                                    ps[:],
                                    ot_sb[t][:, qs * 128 : (qs + 1) * 128],
                                    wo_sb[:, t, dch * 512 : (dch + 1) * 512],
                                    start=(t == 0),
                                    stop=(t == 3),
                                )
                            ost = sp.tile(
                                [128, 512], F32, tag="ost", bufs=3,
                                name=f"os{qc}_{qs}_{dch}",
                            )
                            nc.scalar.copy(ost[:], ps[:])
                            nc.sync.dma_start(
                                out=outp[
                                    q0 + qs * 128 : q0 + (qs + 1) * 128,
                                    dch * 512 : (dch + 1) * 512,
                                ],
                                in_=ost[:],
                            )

    nc.compile()
    return nc


def _get_nc():
    global _NC
    if _NC is None:
        _NC = _build()
    return _NC


def kernel(q, k, v, mask, Wq, bq, Wk, bk, Wv, bv, Wo, bo):
    global LAST_RESULT
    from concourse.bass_utils import run_bass_kernel_spmd

    nc = _get_nc()
    q = np.asarray(q, np.float32)
    k = np.asarray(k, np.float32)
    v = np.asarray(v, np.float32)
    mask = np.asarray(mask)
    Wq = np.asarray(Wq, np.float32)
    Wk = np.asarray(Wk, np.float32)
    Wv = np.asarray(Wv, np.float32)
    Wo = np.asarray(Wo, np.float32)

    negi = np.zeros((128, 128), np.float32)
    np.fill_diagonal(negi, -1e30)
    negi = negi.astype(ml_dtypes.bfloat16)

    qT = [np.ascontiguousarray(q[b].T) for b in range(B)]
    kT = [np.ascontiguousarray(k[b].T) for b in range(B)]
    vT = [np.ascontiguousarray(v[b].T) for b in range(B)]
    cmT = [(mask[b] == 0).T.astype(ml_dtypes.bfloat16) for b in range(B)]

    in_maps = []
    for c in range(8):
        b, half = divmod(c, 2)
        hs = slice(half * 512, (half + 1) * 512)
        in_maps.append(
            dict(
                qt=qT[b],
                kt=kT[b],
                vt=vT[b],
                cmt=cmT[b],
                wq=np.ascontiguousarray(Wq[:, hs]),
                wk=np.ascontiguousarray(Wk[:, hs]),
                wv=np.ascontiguousarray(Wv[:, hs]),
                wo=np.ascontiguousarray(Wo[hs, :]),
                bq=np.ascontiguousarray(
                    np.asarray(bq, np.float32)[hs].reshape(4, 128).T
                ),
                bk=np.ascontiguousarray(
                    np.asarray(bk, np.float32)[hs].reshape(4, 128).T
                ),
                bv=np.asarray(bv, np.float32)[hs].reshape(1, 512),
                negi=negi,
                onescol=np.ones((128, 8, 1), np.float32),
            )
        )

    trace = bool(int(os.environ.get("BASS_KERNEL_TRACE", "0")))
    if trace:
        try:
            try:
                from antenv.axon_hooks import get_axon_ntff_profile_hook
            except ImportError:
                import types

                import antenv

                mod = types.ModuleType("antenv.axon_hooks")
                mod._HOOK = None

                def _set(hook, _m=mod):
                    _m._HOOK = hook

                def _get(_m=mod):
                    return _m._HOOK

                mod.set_axon_ntff_profile_hook = _set
                mod.get_axon_ntff_profile_hook = _get
                sys.modules["antenv.axon_hooks"] = mod
                antenv.axon_hooks = mod
                from antenv.axon_hooks import get_axon_ntff_profile_hook

            if get_axon_ntff_profile_hook() is None:
                if "/root/.axon_site" not in sys.path:
                    sys.path.insert(0, "/root/.axon_site")
                from trn_agent_boot.trn_boot import _ntff_profile_via_ctypes

                sys.modules["antenv.axon_hooks"].set_axon_ntff_profile_hook(
                    _ntff_profile_via_ctypes("/opt/axon/libaxon_pjrt.so")
                )
        except Exception as e:  # tracing is best-effort
            print(f"trace hook setup failed: {e}", file=sys.stderr)
    res = run_bass_kernel_spmd(
        nc, in_maps, core_ids=list(range(8)), trace=trace
    )
    LAST_RESULT = res

    out = np.empty((B, L, D), np.float32)
    attn = np.empty((B, H, L, L), np.float32)
    for c in range(8):
        b, half = divmod(c, 2)
        r = res.results[c]
        attn[b, half * HPC : (half + 1) * HPC] = r["attnT"].transpose(0, 2, 1)
        if half == 0:
            out[b] = r["outp"]
        else:
            out[b] += r["outp"]
    out += np.asarray(bo, np.float32)
    return out, attn


# revision 10
# speedup vs baseline: 1.0688x; 1.0055x over previous
"""Self-contained Trainium2 Bass kernel for nn_MultiHeadAttention.

Strategy (8 cores = 4 batches x 2 head-halves, 8 heads/core):
 - Everything computed in transposed "S^T" layout [k, q] so no on-chip
   transposes are needed anywhere:
     Q^T,K^T [dk, L] projections from host-transposed q^T,k^T inputs
     V [L, dv] projection from host-transposed v^T (lhsT = v^T blocks)
     S^T tile = (K^T block)^T @ Q^T chunk        (C=64, N=512, float32r)
     mask:  S^T += (-1e30*I) @ cmT  via PSUM accumulation (cm = 1-mask^T, bf16)
     Em^T  = exp(S^T * 1/8)  on ScalarE (masked entries -> exact 0)
     O^T  += [V_h | 1]^T @ Em^T  -> row 64 = softmax denominators (free)
     r = 1/sums;  R = ones^T @ r broadcast;  P^T = Em^T * R (DVE, in place)
     out_p = (O^T_norm blocks)^T @ Wo_half     (partial, host sums halves)
 - attn is written transposed [h, k, q] per core; host transposes back.
"""

import os
import sys

sys.path.insert(0, "/opt/trn_rl_repo")

import numpy as np
import ml_dtypes

B, L, D = 4, 2048, 1024
H, DK, DV = 16, 64, 64
HPC = 8  # heads per core
QC = 512  # q-chunk size
NQC = L // QC  # 4
NKT = L // 128  # 16 k-tiles

_NC = None
LAST_RESULT = None


def _build():
    import concourse.tile as tile
    from concourse import bacc, mybir

    F32 = mybir.dt.float32
    F32R = mybir.dt.float32r
    BF16 = mybir.dt.bfloat16
    Act = mybir.ActivationFunctionType

    nc = bacc.Bacc(None, target_bir_lowering=False)

    qt = nc.declare_dram_parameter("qt", [D, L], F32R, isOutput=False)
    kt = nc.declare_dram_parameter("kt", [D, L], F32R, isOutput=False)
    vt = nc.declare_dram_parameter("vt", [D, L], F32R, isOutput=False)
    cmt = nc.declare_dram_parameter("cmt", [L, L], BF16, isOutput=False)
    wq = nc.declare_dram_parameter("wq", [D, 512], F32R, isOutput=False)
    wk = nc.declare_dram_parameter("wk", [D, 512], F32R, isOutput=False)
    wv = nc.declare_dram_parameter("wv", [D, 512], F32R, isOutput=False)
    wo = nc.declare_dram_parameter("wo", [512, D], F32R, isOutput=False)
    bq = nc.declare_dram_parameter("bq", [128, 4], F32, isOutput=False)
    bk = nc.declare_dram_parameter("bk", [128, 4], F32, isOutput=False)
    bv = nc.declare_dram_parameter("bv", [1, 512], F32, isOutput=False)
    negi = nc.declare_dram_parameter("negi", [128, 128], BF16, isOutput=False)
    onescol = nc.declare_dram_parameter("onescol", [128, 8, 1], F32R, isOutput=False)

    attnT = nc.declare_dram_parameter("attnT", [HPC, L, L], F32, isOutput=True)
    outp = nc.declare_dram_parameter("outp", [L, D], F32, isOutput=True)

    with tile.TileContext(nc) as tc:
        with (
            tc.tile_pool(name="persist", bufs=1) as pp,
            tc.tile_pool(name="const", bufs=1) as cp,
        ):
            qt_sb = [pp.tile([128, L], F32R, name=f"qt_sb{m}") for m in range(4)]
            kt_sb = [pp.tile([128, L], F32R, name=f"kt_sb{m}") for m in range(4)]
            vext = [pp.tile([128, HPC, 65], F32R, name=f"vext{t}") for t in range(16)]
            ones_sb = cp.tile([1, 128], F32)
            nc.vector.memset(ones_sb[:], 1.0)
            negi_sb = cp.tile([128, 128], BF16)
            nc.sync.dma_start(out=negi_sb[:], in_=negi[:])

            # ---------------- Phase A: projections ----------------
            with (
                tc.tile_pool(name="wpool", bufs=1) as wp,
                tc.tile_pool(name="xstream", bufs=2) as xp,
                tc.tile_pool(name="biasp", bufs=1) as bp,
                tc.tile_pool(name="psA", bufs=4, space="PSUM") as psA,
            ):
                wq_sb = wp.tile([128, 8, 512], F32R)
                nc.sync.dma_start(
                    out=wq_sb[:], in_=wq[:].rearrange("(t p) j -> p t j", p=128)
                )
                wk_sb = wp.tile([128, 8, 512], F32R)
                nc.sync.dma_start(
                    out=wk_sb[:], in_=wk[:].rearrange("(t p) j -> p t j", p=128)
                )
                wv_sb = wp.tile([128, 8, 512], F32R)
                nc.sync.dma_start(
                    out=wv_sb[:], in_=wv[:].rearrange("(t p) j -> p t j", p=128)
                )
                bq_sb = bp.tile([128, 4], F32)
                nc.sync.dma_start(out=bq_sb[:], in_=bq[:])
                bk_sb = bp.tile([128, 4], F32)
                nc.sync.dma_start(out=bk_sb[:], in_=bk[:])
                bv_sb = bp.tile([1, 512], F32)
                nc.sync.dma_start(out=bv_sb[:], in_=bv[:])

                for src, wsb, bsb, dst in (
                    (qt, wq_sb, bq_sb, qt_sb),
                    (kt, wk_sb, bk_sb, kt_sb),
                ):
                    for l in range(4):
                        xs = xp.tile([128, 8, 512], F32R, tag="xs", name=f"xs_{l}")
                        nc.sync.dma_start(
                            out=xs[:],
                            in_=src[:].rearrange("(t p) j -> p t j", p=128)[
                                :, :, l * 512 : (l + 1) * 512
                            ],
                        )
                        for m in range(4):
                            ps = psA.tile([128, 512], F32, tag="pa", name=f"pa{l}{m}")
                            for t in range(8):
                                nc.tensor.matmul(
                                    ps[:],
                                    wsb[:, t, m * 128 : (m + 1) * 128],
                                    xs[:, t, :],
                                    start=(t == 0),
                                    stop=(t == 7),
                                )
                            nc.scalar.activation(
                                dst[m][:, l * 512 : (l + 1) * 512],
                                ps[:],
                                Act.Identity,
                                bias=bsb[:, m : m + 1],
                            )
                # V projection: natural [L, dv] with ones column per head
                for l in range(4):
                    xs = xp.tile([128, 8, 512], F32R, tag="xs", name=f"xsv_{l}")
                    nc.sync.dma_start(
                        out=xs[:],
                        in_=vt[:].rearrange("(t p) j -> p t j", p=128)[
                            :, :, l * 512 : (l + 1) * 512
                        ],
                    )
                    for ls in range(4):
                        lt = l * 4 + ls
                        ps = psA.tile([128, 512], F32, tag="pa", name=f"pv{lt}")
                        for t in range(8):
                            nc.tensor.matmul(
                                ps[:],
                                xs[:, t, ls * 128 : (ls + 1) * 128],
                                wv_sb[:, t, :],
                                start=(t == 0),
                                stop=False,
                            )
                        nc.tensor.matmul(
                            ps[:], ones_sb[:], bv_sb[:], start=False, stop=True
                        )
                        nc.sync.dma_start(
                            out=vext[lt][:, :, 64:65],
                            in_=onescol[:],
                        )
                        for h in range(HPC):
                            nc.scalar.copy(
                                vext[lt][:, h, 0:64], ps[:, h * 64 : (h + 1) * 64]
                            )

            tc.strict_bb_all_engine_barrier()

            # ---------------- Phase B: attention ----------------
            with (
                tc.tile_pool(name="empool", bufs=5) as ep,
                tc.tile_pool(name="cmpool", bufs=4) as cmp_,
                tc.tile_pool(name="otpool", bufs=2) as op_,
                tc.tile_pool(name="wopool", bufs=1) as wop,
                tc.tile_pool(name="smallp", bufs=2) as sp,
                tc.tile_pool(name="psS", bufs=2, space="PSUM") as psS,
                tc.tile_pool(name="psO", bufs=2, space="PSUM") as psO,
                tc.tile_pool(name="psR", bufs=1, space="PSUM") as psR,
                tc.tile_pool(name="psOP", bufs=1, space="PSUM") as psOP,
            ):
                wo_sb = wop.tile([128, 4, D], F32R)
                nc.sync.dma_start(
                    out=wo_sb[:], in_=wo[:].rearrange("(t p) j -> p t j", p=128)
                )
                for qc in range(NQC):
                    q0 = qc * QC
                    cm_t = []
                    for g in range(4):
                        t_ = cmp_.tile(
                            [128, 4, QC], BF16, tag="cm", name=f"cm_{qc}_{g}"
                        )
                        nc.sync.dma_start(
                            out=t_[:],
                            in_=cmt[:].rearrange("(kt p) j -> p kt j", p=128)[
                                :, 4 * g : 4 * g + 4, q0 : q0 + QC
                            ],
                        )
                        cm_t.append(t_)
                    ot_sb = [
                        op_.tile([128, QC], F32R, tag=f"otsb{t}", name=f"ot_{qc}_{t}")
                        for t in range(4)
                    ]
                    for h in range(HPC):
                        m, r0 = h // 2, (h % 2) * 64
                        ot_ps = psO.tile([65, QC], F32, tag="ot", name=f"otp_{qc}_{h}")
                        em_t = [
                            ep.tile([128, 2048], F32R, tag="em", name=f"em{qc}_{h}_{g}")
                            for g in range(4)
                        ]
                        for jj in range(8):  # kt pairs
                            s_ps = psS.tile(
                                [128, 1024], F32, tag="s", name=f"s{qc}_{h}_{jj}"
                            )
                            for i in range(2):
                                ikt = jj * 2 + i
                                sl = s_ps[:, i * 512 : (i + 1) * 512]
                                nc.tensor.matmul(
                                    sl,
                                    negi_sb[:],
                                    cm_t[ikt // 4][:, ikt % 4, :],
                                    start=True,
                                    stop=False,
                                )
                                nc.tensor.matmul(
                                    sl,
                                    kt_sb[m][r0 : r0 + 64, ikt * 128 : (ikt + 1) * 128],
                                    qt_sb[m][r0 : r0 + 64, q0 : q0 + QC],
                                    start=False,
                                    stop=True,
                                )
                            g, hf = jj // 2, jj % 2
                            emsl = em_t[g][:, hf * 1024 : (hf + 1) * 1024]
                            nc.scalar.activation(emsl, s_ps[:], Act.Exp, scale=0.125)
                            for i in range(2):
                                ikt = jj * 2 + i
                                nc.tensor.matmul(
                                    ot_ps[:],
                                    vext[ikt][:, h, :],
                                    em_t[g][:, (2 * hf + i) * 512 : (2 * hf + i + 1) * 512],
                                    start=(ikt == 0),
                                    stop=(ikt == NKT - 1),
                                )
                        sums_sb = sp.tile([1, QC], F32, tag="r", name=f"r{qc}_{h}")
                        nc.scalar.copy(sums_sb[:], ot_ps[64:65, :])
                        rbc = psR.tile([128, QC], F32, tag="rbc", name=f"rb{qc}_{h}")
                        nc.tensor.matmul(
                            rbc[:], ones_sb[:], sums_sb[:], start=True, stop=True
                        )
                        rbc_sb = sp.tile([128, QC], F32, tag="rsb", name=f"rs{qc}_{h}")
                        nc.vector.reciprocal_approx_fast(rbc_sb[:], rbc[:])
                        # normalize O^T into ot_sb (f32r out)
                        nc.vector.tensor_mul(
                            ot_sb[m][r0 : r0 + 64, :], ot_ps[0:64, :], rbc_sb[0:64, :]
                        )
                        # normalize P^T in place, then DMA to attnT
                        for g in range(4):
                            for i in range(4):
                                nc.vector.tensor_mul(
                                    em_t[g][:, i * 512 : (i + 1) * 512],
                                    em_t[g][:, i * 512 : (i + 1) * 512].bitcast(F32),
                                    rbc_sb[:],
                                )
                            nc.sync.dma_start(
                                out=attnT[h].rearrange("(kt p) j -> p kt j", p=128)[
                                    :, 4 * g : 4 * g + 4, q0 : q0 + QC
                                ],
                                in_=em_t[g][:]
                                .bitcast(F32)
                                .rearrange("p (kt j) -> p kt j", j=512),
                            )
                    # out projection for this q-chunk (partial out, bo on host)
                    for qs in range(4):
                        for dch in range(2):
                            ps = psOP.tile(
                                [128, 512], F32, tag="sop", name=f"op{qc}_{qs}_{dch}"
                            )
                            for t in range(4):
                                nc.tensor.matmul(
                                    ps[:],
                                    ot_sb[t][:, qs * 128 : (qs + 1) * 128],
                                    wo_sb[:, t, dch * 512 : (dch + 1) * 512],
                                    start=(t == 0),
                                    stop=(t == 3),
                                )
                            ost = sp.tile(
                                [128, 512], F32, tag="ost", bufs=3,
                                name=f"os{qc}_{qs}_{dch}",
                            )
                            nc.scalar.copy(ost[:], ps[:])
                            nc.sync.dma_start(
                                out=outp[
                                    q0 + qs * 128 : q0 + (qs + 1) * 128,
                                    dch * 512 : (dch + 1) * 512,
                                ],
                                in_=ost[:],
                            )

    nc.compile()
    return nc


def _get_nc():
    global _NC
    if _NC is None:
        _NC = _build()
    return _NC


def kernel(q, k, v, mask, Wq, bq, Wk, bk, Wv, bv, Wo, bo):
    global LAST_RESULT
    from concourse.bass_utils import run_bass_kernel_spmd

    nc = _get_nc()
    q = np.asarray(q, np.float32)
    k = np.asarray(k, np.float32)
    v = np.asarray(v, np.float32)
    mask = np.asarray(mask)
    Wq = np.asarray(Wq, np.float32)
    Wk = np.asarray(Wk, np.float32)
    Wv = np.asarray(Wv, np.float32)
    Wo = np.asarray(Wo, np.float32)

    negi = np.zeros((128, 128), np.float32)
    np.fill_diagonal(negi, -1e30)
    negi = negi.astype(ml_dtypes.bfloat16)

    qT = [np.ascontiguousarray(q[b].T) for b in range(B)]
    kT = [np.ascontiguousarray(k[b].T) for b in range(B)]
    vT = [np.ascontiguousarray(v[b].T) for b in range(B)]
    cmT = [(mask[b] == 0).T.astype(ml_dtypes.bfloat16) for b in range(B)]

    in_maps = []
    for c in range(8):
        b, half = divmod(c, 2)
        hs = slice(half * 512, (half + 1) * 512)
        in_maps.append(
            dict(
                qt=qT[b],
                kt=kT[b],
                vt=vT[b],
                cmt=cmT[b],
                wq=np.ascontiguousarray(Wq[:, hs]),
                wk=np.ascontiguousarray(Wk[:, hs]),
                wv=np.ascontiguousarray(Wv[:, hs]),
                wo=np.ascontiguousarray(Wo[hs, :]),
                bq=np.ascontiguousarray(
                    np.asarray(bq, np.float32)[hs].reshape(4, 128).T
                ),
                bk=np.ascontiguousarray(
                    np.asarray(bk, np.float32)[hs].reshape(4, 128).T
                ),
                bv=np.asarray(bv, np.float32)[hs].reshape(1, 512),
                negi=negi,
                onescol=np.ones((128, 8, 1), np.float32),
            )
        )

    trace = bool(int(os.environ.get("BASS_KERNEL_TRACE", "0")))
    if trace:
        try:
            try:
                from antenv.axon_hooks import get_axon_ntff_profile_hook
            except ImportError:
                import types

                import antenv

                mod = types.ModuleType("antenv.axon_hooks")
                mod._HOOK = None

                def _set(hook, _m=mod):
                    _m._HOOK = hook

                def _get(_m=mod):
                    return _m._HOOK

                mod.set_axon_ntff_profile_hook = _set
                mod.get_axon_ntff_profile_hook = _get
                sys.modules["antenv.axon_hooks"] = mod
                antenv.axon_hooks = mod
                from antenv.axon_hooks import get_axon_ntff_profile_hook

            if get_axon_ntff_profile_hook() is None:
                if "/root/.axon_site" not in sys.path:
                    sys.path.insert(0, "/root/.axon_site")
                from trn_agent_boot.trn_boot import _ntff_profile_via_ctypes

                sys.modules["antenv.axon_hooks"].set_axon_ntff_profile_hook(
                    _ntff_profile_via_ctypes("/opt/axon/libaxon_pjrt.so")
                )
        except Exception as e:  # tracing is best-effort
            print(f"trace hook setup failed: {e}", file=sys.stderr)
    res = run_bass_kernel_spmd(
        nc, in_maps, core_ids=list(range(8)), trace=trace
    )
    LAST_RESULT = res

    out = np.empty((B, L, D), np.float32)
    attn = np.empty((B, H, L, L), np.float32)
    for c in range(8):
        b, half = divmod(c, 2)
        r = res.results[c]
        attn[b, half * HPC : (half + 1) * HPC] = r["attnT"].transpose(0, 2, 1)
        if half == 0:
            out[b] = r["outp"]
        else:
            out[b] += r["outp"]
    out += np.asarray(bo, np.float32)
    return out, attn
